# revision 7
# baseline (speedup 1.0000x reference)
"""Trainium2 Bass kernel for SimCLR NT-Xent contrastive loss (N=4096, D=512, T=0.5).

Math: with z = rownorm(concat(emb_i, emb_j)) (8192x512) and S = z @ z.T:
  loss = (1/2N) * [ sum_r log(rowsum_r(exp(S/T)) - exp(1/T)) - (1/T) * sum_r S[r, (r+N) mod 2N] ]

Distribution: data-parallel over rows of z. Each of the 8 cores receives a
block-rotated copy of the concatenated input (rotation by 1024*c rows), so the
same program computes row block [0:1024) of its rotated similarity matrix
against all 8192 columns. Rotation preserves both the row set (each original
row handled exactly once across cores) and the +N pair structure (mod 2N).

Per-core pipeline (all compute on device):
  1. load raw f32 rows -> SBUF
  2. rownorm: square+row-sum (DVE, fused accum) -> rsqrt via Quake seed + 2
     Newton steps (DVE only; avoids ACT sqrt<->exp table thrash)
  3. scale rows by 1/norm, cast bf16
  4. bf16 z roundtrip through DRAM + xbar DMA-transpose -> zT [d, rows] in SBUF
  5. 512x bf16 matmul (128x128x512) into PSUM; ACT exp(2x) with fused row-sum
  6. log(denom) via single Ln at the end; positives via fused mul+row-sum on rows
Host merges 8 partial [128,8] tensors (log-denoms, pair-dots) into the scalar.
"""

import numpy as np

for _p in ("/opt/trn_rl_repo", "/root/.axon_site/_ro/trn_rl_repo"):
    try:
        import concourse  # noqa: F401
        break
    except ImportError:
        import sys
        if _p not in sys.path:
            sys.path.insert(0, _p)

import concourse.bass as bass
import concourse.bacc as bacc
import concourse.tile as tile
from concourse import mybir
from concourse.bass_utils import run_bass_kernel_spmd

F32 = mybir.dt.float32
I32 = mybir.dt.int32
BF16 = mybir.dt.bfloat16
ALU = mybir.AluOpType
AF = mybir.ActivationFunctionType

N_CORES = 8
BATCH = 4096
DIM = 512
ROWS = 2 * BATCH            # 8192
BLOCK = ROWS // N_CORES     # 1024 rows per core
P = 128                     # partitions
NT = ROWS // P              # 64 row tiles
NG = 16                     # load groups (4 row-tiles each)
TPG = 4                     # tiles per group
RG = 4                      # DRAM scratch row-ranges (2048 rows each)
KC = DIM // P               # 4 k-chunks
MT = BLOCK // P             # 8 m-tiles
CG = 4                      # column groups of 2048
CGW = ROWS // CG            # 2048
NW = 512                    # matmul free width
TEMP_SCALE = 2.0            # 1/T
MAGIC = 0x5F3759DF


def _build_program():
    nc = bacc.Bacc(trn_type="TRN2")
    x_in = nc.declare_dram_parameter("x", [ROWS, DIM], F32, isOutput=False)
    logd_out = nc.declare_dram_parameter("logd", [P, MT], F32, isOutput=True)
    pos_out = nc.declare_dram_parameter("pos", [P, MT], F32, isOutput=True)

    with tile.TileContext(nc) as tc:
        with tc.tile_pool(name="xg", bufs=3) as xg_pool, \
             tc.tile_pool(name="zbig", bufs=3) as zbig_pool, \
             tc.tile_pool(name="sq", bufs=2) as sq_pool, \
             tc.tile_pool(name="small", bufs=2) as small_pool, \
             tc.tile_pool(name="single", bufs=1) as singles, \
             tc.tile_pool(name="zt", bufs=1) as zt_pool, \
             tc.tile_pool(name="escr", bufs=2) as e_pool, \
             tc.tile_pool(name="accp", bufs=2) as acc_pool, \
             tc.tile_pool(name="zdram", bufs=1, space="DRAM") as dram_pool, \
             tc.tile_pool(name="psum", bufs=2, space="PSUM") as psum_pool:

            n2 = singles.tile([P, NT], F32, tag="n2")
            inv = singles.tile([P, NT], F32, tag="inv")
            magic4 = singles.tile([P, TPG], I32, tag="magic4")
            nc.vector.memset(magic4, MAGIC)
            pos_acc = singles.tile([P, MT], F32, tag="pos_acc")
            den_all = singles.tile([P, MT], F32, tag="den_all")
            lnbias = singles.tile([P, 1], F32, tag="lnbias")
            nc.vector.memset(lnbias, -float(np.exp(2.0)))

            zd = [dram_pool.tile([ROWS // RG, DIM], BF16, tag=f"zd{r}", name=f"zd{r}")
                  for r in range(RG)]
            # zT[k][rg]: [128 (d-chunk k), 2048 (rows rg)] bf16
            zT = [[zt_pool.tile([P, CGW], BF16, tag=f"zt_{k}_{r}", name=f"zt_{k}_{r}")
                   for r in range(RG)] for k in range(KC)]

            zbigs = {}  # row-range idx -> assembled bf16 z tile [P, 16, DIM]

            # ---- Phase 1: normalize rows, write bf16 z to DRAM scratch ----
            for g in range(NG):
                r0 = g * TPG * P  # 512 rows per group
                xg = xg_pool.tile([P, TPG, DIM], F32, tag="xg")
                nc.sync.dma_start(
                    out=xg,
                    in_=x_in[r0:r0 + TPG * P, :].rearrange("(a p) d -> p a d", p=P))
                for a in range(TPG):
                    sq = sq_pool.tile([P, DIM], F32, tag="sq")
                    nc.vector.scalar_tensor_tensor(
                        out=sq, in0=xg[:, a, :], scalar=0.0, in1=xg[:, a, :],
                        op0=ALU.bypass, op1=ALU.mult,
                        accum_out=n2[:, g * TPG + a: g * TPG + a + 1])
                # rsqrt on this group's 4 norms: Quake seed + 2 Newton steps
                sl = n2[:, g * TPG:(g + 1) * TPG]
                isl = inv[:, g * TPG:(g + 1) * TPG]
                sh = small_pool.tile([P, TPG], I32, tag="sh")
                nc.vector.tensor_scalar(
                    out=sh, in0=sl.bitcast(I32), scalar1=1, scalar2=None,
                    op0=ALU.logical_shift_right)
                seed = small_pool.tile([P, TPG], I32, tag="seed")
                nc.vector.scalar_tensor_tensor(
                    out=seed, in0=magic4, scalar=0.0, in1=sh,
                    op0=ALU.bypass, op1=ALU.subtract)
                y = seed.bitcast(F32)
                for it in range(2):
                    ta = small_pool.tile([P, TPG], F32, tag="ta")
                    tb = small_pool.tile([P, TPG], F32, tag="tb")
                    nc.vector.tensor_mul(out=ta, in0=y, in1=y)
                    nc.vector.scalar_tensor_tensor(
                        out=tb, in0=ta, scalar=-0.5, in1=sl,
                        op0=ALU.mult, op1=ALU.mult)
                    nc.vector.tensor_scalar(
                        out=tb, in0=tb, scalar1=1.5, scalar2=None, op0=ALU.add)
                    dst = isl if it == 1 else y
                    nc.vector.tensor_mul(out=dst, in0=y, in1=tb)

                rr = g // 4
                if g % 4 == 0:
                    zbigs[rr] = zbig_pool.tile(
                        [P, 4 * TPG, DIM], BF16, tag="zbig", name=f"zbig{rr}")
                zb = zbigs[rr]
                jlo = (g % 4) * TPG
                for a in range(TPG):
                    nc.vector.tensor_scalar_mul(
                        out=zb[:, jlo + a, :], in0=xg[:, a, :],
                        scalar1=inv[:, g * TPG + a: g * TPG + a + 1])
                # positive pairs: rotated rows [0:1024) pair with [4096:5120)
                # i.e. tiles 0..7 (range 0 slices 0..7) with tiles 32..39
                # (range 2 slices 0..7)
                if g in (8, 9):
                    slo = (g - 8) * TPG
                    for a in range(TPG):
                        psc = sq_pool.tile([P, DIM], BF16, tag="psc")
                        nc.vector.scalar_tensor_tensor(
                            out=psc, in0=zbigs[0][:, slo + a, :], scalar=0.0,
                            in1=zb[:, jlo + a, :], op0=ALU.bypass, op1=ALU.mult,
                            accum_out=pos_acc[:, slo + a: slo + a + 1])
                if g % 4 == 3:
                    # single 2 MB writer per row-range: the downstream xbar
                    # transpose has very few sync-wait slots, so it must
                    # depend on exactly one DMA
                    nc.sync.dma_start(
                        out=zd[rr][:, :].rearrange("(s p) d -> p s d", p=P),
                        in_=zb)

            # ---- Phase 2: xbar transpose bf16 z -> zT ----
            for rr in range(RG):
                for k in range(KC):
                    nc.sync.dma_start_transpose(
                        out=zT[k][rr], in_=zd[rr][:, k * P:(k + 1) * P])

            # ---- Phase 3: row-block x all-columns matmul, exp row-sums ----
            for m in range(MT):
                accm = acc_pool.tile([P, CG], F32, tag="accm")
                for cg in range(CG):
                    ps = psum_pool.tile([P, CGW], F32, tag="ps")
                    for n in range(CGW // NW):
                        for k in range(KC):
                            nc.tensor.matmul(
                                ps[:, n * NW:(n + 1) * NW],
                                lhsT=zT[k][0][:, m * P:(m + 1) * P],
                                rhs=zT[k][cg][:, n * NW:(n + 1) * NW],
                                start=(k == 0), stop=(k == KC - 1))
                    e_scr = e_pool.tile([P, CGW], BF16, tag="escr")
                    nc.scalar.activation(
                        out=e_scr, in_=ps, func=AF.Exp, scale=TEMP_SCALE,
                        accum_out=accm[:, cg:cg + 1])
                nc.vector.reduce_sum(
                    out=den_all[:, m:m + 1], in_=accm,
                    axis=mybir.AxisListType.X)

            # ---- Phase 4: log-denoms, outputs ----
            logd = singles.tile([P, MT], F32, tag="logd")
            nc.scalar.activation(out=logd, in_=den_all, func=AF.Ln,
                                 bias=lnbias, scale=1.0)
            nc.sync.dma_start(out=logd_out[:, :], in_=logd)
            nc.sync.dma_start(out=pos_out[:, :], in_=pos_acc)

    nc.finalize()
    return nc


_CACHE = {}


def _run(full: np.ndarray, trace: bool = False, **kwargs):
    """Run the SPMD program on all 8 cores; returns BassKernelResults."""
    if "nc" not in _CACHE:
        _CACHE["nc"] = _build_program()
    nc = _CACHE["nc"]
    in_maps = [
        {"x": np.ascontiguousarray(np.roll(full, -BLOCK * c, axis=0))}
        for c in range(N_CORES)
    ]
    return run_bass_kernel_spmd(
        nc, in_maps, core_ids=list(range(N_CORES)), trace=trace, **kwargs)


def _merge(results) -> np.ndarray:
    logd_sum = 0.0
    pos_sum = 0.0
    for r in results:
        logd_sum += r["logd"].astype(np.float64).sum()
        pos_sum += r["pos"].astype(np.float64).sum()
    loss = (logd_sum - TEMP_SCALE * pos_sum) / (2.0 * BATCH)
    return np.array(loss, dtype=np.float32)


def kernel(emb_i: np.ndarray, emb_j: np.ndarray) -> np.ndarray:
    full = np.concatenate(
        [np.asarray(emb_i, np.float32), np.asarray(emb_j, np.float32)], axis=0)
    return _merge(_run(full).results)
